# revision 13
# baseline (speedup 1.0000x reference)
"""FANMoE HyperNet layer on 8 TRN2 NeuronCores — pair-sharded, y-stationary.

Sharding: 4 expert-pairs x 2 batch-halves. Each core owns 2 experts
(W = 2*(DP+DN) = 384 output cols) and 256 samples, streaming its fp16
hW2 hi-slice (12.6MB) from HBM once per call.

Key structure: the per-k combine out += h[:,k] * (x @ W_k) is folded into
the matmul by scaling the stationary operand: y_k[i,b] = x_hi[i,b]*h[k,b]
(fp16, built on DVE with broadcast APs from a partition-broadcast of h).
PSUM then accumulates all 64 k plus the base/bias terms in one
accumulation group per batch tile — no per-k elementwise combine exists
(measured: any such combine costs >=51us on the fastest engine).

Measured-engine budget per core (main loop): PE 256 mms x 384 rows x
165ns = 42us; DMA weights 12.6MB ~ 49us (the wall); DVE grouped y-build
~19us; h-broadcast split Pool (757ns/k) / PE+Act ones-matmul path.

Accuracy: 1-term fp16 (x_hi, h_16, W_hi, y rounded) gives ~2^-11-relative
matmul error => phase error ~4e-3 rms on per-values, ~5e-3 relative
output error against the 2e-2 gate.
"""
import math

import numpy as np

import concourse.bass as bass
import concourse.tile as tile
from concourse import mybir, bacc
from concourse.masks import make_identity

B, IN, OUT, COND, N, H = 512, 256, 256, 128, 8, 64
DP = 64
DN = 128
TPE = IN * DP + IN * DN + DN
BH = B // 2          # samples per core (batch half)
NBT = BH // 128      # 2 tiles of 128 rows
WE = DP + DN         # per-expert width 192
W = 2 * WE           # per-core output width 384
dt = mybir.dt
F32 = dt.float32
F16 = dt.float16
AF = mybir.ActivationFunctionType
OP = mybir.AluOpType
INV2PI = 1.0 / (2.0 * math.pi)
N2PI = -2.0 * math.pi

_cache = {}

KG = 8        # k-group size (DMA/y-build granularity)
NPOOL = 6     # of each KG: first NPOOL h-broadcasts on Pool, rest PE+Act
DMASPLIT = 1  # weight-tile DMAs issued per k (1 or 2 halves)


def _build(terms=1, repeat_main=1, ablate=(), kg=None, npool=None,
           dmasplit=None):
    kg = KG if kg is None else kg
    npool = NPOOL if npool is None else npool
    dmasplit = DMASPLIT if dmasplit is None else dmasplit
    nc = bacc.Bacc("TRN2", target_bir_lowering=False, debug=False)

    def din(name, shape, dty=F32):
        return nc.dram_tensor(name, shape, dty, kind="ExternalInput").ap()

    xT32 = din("xT32", (2, 128, BH))
    xh16 = din("xh16", (2, 128, BH), F16)
    w2h = din("w2h", (H, 128, 2 * W), F16)
    w2b = din("w2b", (H + 1, 2, DN), F16)
    wbase = din("wbase", (2, 128, W))
    condT = din("condT", (COND, BH), F16)
    hW1 = din("hW1", (COND, H), F16)
    hb1 = din("hb1", (1, H), F16)
    gW1 = din("gW1", (COND, 3 * N), F16)
    gb1 = din("gb1", (3 * N, 1))
    gW2 = din("gW2", (3 * N, N), F16)
    gb2 = din("gb2", (1, N), F16)
    out = nc.dram_tensor("out", (BH, OUT), F32, kind="ExternalOutput").ap()

    with tile.TileContext(nc) as tc:
        with tc.tile_pool(name="const", bufs=1) as cp, \
             tc.tile_pool(name="tmp", bufs=4) as tp, \
             tc.tile_pool(name="acc", bufs=1, space="PSUM") as ap_:
            ident = cp.tile([128, 128], F16)
            make_identity(nc, ident)
            ones16 = cp.tile([1, 128], F16)
            nc.vector.memset(ones16, 1.0)
            zc = cp.tile([1, W], F16)
            nc.vector.memset(zc, 0.0)

            psA = [ap_.tile([128, W], F32, tag=f"psA{t}", name=f"psA{t}",
                            bufs=1) for t in range(NBT)]

            sxT = cp.tile([128, 2, BH], F32)
            sxh = cp.tile([128, 2, BH], F16)
            for c in range(2):
                nc.sync.dma_start(sxT[:, c, :], xT32[c])
                nc.sync.dma_start(sxh[:, c, :], xh16[c])
            scT = cp.tile([COND, BH], F16)
            nc.sync.dma_start(scT, condT)
            swb = cp.tile([128, 2, W], F32)
            for c in range(2):
                nc.sync.dma_start(swb[:, c, :], wbase[c])
            sw2b = cp.tile([H + 1, 2, DN], F16)
            for e in range(2):
                nc.sync.dma_start(sw2b[:, e, :], w2b[:, e])
            shW1 = cp.tile([COND, H], F16)
            nc.sync.dma_start(shW1, hW1)
            shb1 = cp.tile([1, H], F16)
            nc.sync.dma_start(shb1, hb1)
            sgW1 = cp.tile([COND, 3 * N], F16)
            nc.sync.dma_start(sgW1, gW1)
            sgb1 = cp.tile([3 * N, 1], F32)
            nc.sync.dma_start(sgb1, gb1)
            sgW2 = cp.tile([3 * N, N], F16)
            nc.sync.dma_start(sgW2, gW2)
            sgb2 = cp.tile([1, N], F16)
            nc.sync.dma_start(sgb2, gb2)

            hTa = cp.tile([H + 1, BH], F16)
            nc.vector.memset(hTa[H:H + 1, :], 1.0)
            hTf = cp.tile([1, H * BH], F16)
            h16 = [cp.tile([128, H], F16, name=f"h{t}") for t in range(NBT)]
            outf = [cp.tile([128, OUT], F32, name=f"of{t}") for t in range(NBT)]
            gw_sb = [cp.tile([128, 2], F32, name=f"gw{t}") for t in range(NBT)]

            # ---------------- prologue: gating, hypernet h, base ----------
            with tc.tile_pool(name="pps", bufs=2, space="PSUM") as pps:
                g1 = pps.tile([3 * N, BH], F32, tag="g1", bufs=1)
                nc.tensor.matmul(g1, sgW1, scT, start=True, stop=True)
                g1s = cp.tile([3 * N, BH], F16)
                nc.scalar.activation(g1s, g1, AF.Relu, bias=sgb1)

                for bt in range(NBT):
                    bs = slice(bt * 128, bt * 128 + 128)
                    hp = pps.tile([128, H], F32, tag="hp", bufs=1)
                    nc.tensor.matmul(hp, scT[:, bs], shW1,
                                     start=True, stop=False)
                    nc.tensor.matmul(hp, ones16, shb1, start=False, stop=True)
                    nc.scalar.activation(h16[bt], hp, AF.Relu)
                    ht = pps.tile([H, 128], F16, tag="ht", bufs=1)
                    nc.tensor.transpose(ht, h16[bt], ident)
                    nc.scalar.copy(hTa[0:H, bs], ht)

                    lg = pps.tile([128, N], F32, tag="lg", bufs=1)
                    nc.tensor.matmul(lg, g1s[:, bs], sgW2,
                                     start=True, stop=False)
                    nc.tensor.matmul(lg, ones16, sgb2, start=False, stop=True)
                    nmx = tp.tile([128, 1], F32, tag="nmx")
                    nc.vector.tensor_reduce(nmx, lg, axis=mybir.AxisListType.X,
                                            op=OP.max, negate=True)
                    ex = tp.tile([128, N], F32, tag="ex")
                    nc.scalar.activation(ex, lg, AF.Exp, bias=nmx)
                    sm = tp.tile([128, 1], F32, tag="sm")
                    nc.vector.tensor_reduce(sm, ex, axis=mybir.AxisListType.X,
                                            op=OP.add)
                    rv = tp.tile([128, 1], F32, tag="rv")
                    nc.vector.reciprocal(rv, sm)
                    nc.vector.tensor_scalar_mul(gw_sb[bt], ex[:, 0:2], rv)

                    # open the psA accumulation group: base + biases
                    nc.tensor.matmul(psA[bt], sxT[:, 0, bs], swb[:, 0, :],
                                     start=True, stop=False,
                                     skip_group_check=True)
                    nc.tensor.matmul(psA[bt], sxT[:, 1, bs], swb[:, 1, :],
                                     start=False, stop=False,
                                     skip_group_check=True)
                    for e in range(2):
                        cs = slice(e * WE + DP, (e + 1) * WE)
                        nc.tensor.matmul(psA[bt][:, cs], hTa[:, bs],
                                         sw2b[:, e, :], start=False,
                                         stop=False, skip_group_check=True)

                # h rows -> one partition-0 row for partition_broadcast
                for k in range(H):
                    nc.sync.dma_start(hTf[0:1, k * BH:(k + 1) * BH],
                                      hTa[k:k + 1, :])

            # ---------------- main loop over hypernet units k --------------
            with tc.tile_pool(name="wp", bufs=3 * kg) as wp, \
                 tc.tile_pool(name="yb", bufs=2) as yb, \
                 tc.tile_pool(name="mps", bufs=2, space="PSUM") as mps:

              wconst = None
              if "dma" in ablate:
                  wconst = [cp.tile([128, 2, W], F16, name=f"wc{i}")
                            for i in range(kg)]
                  for i in range(kg):
                      nc.sync.dma_start(wconst[i], w2h[i])
              yconst = hconst = None
              if "ybuild" in ablate:
                  yconst = cp.tile([128, kg, 2, 128 * NBT], F16, name="yc")
                  nc.vector.memset(yconst, 0.25)

              def _main_body():
                for k0 in range(0, H, kg):
                    if "dma" in ablate:
                        wts = wconst
                    else:
                        wts = []
                        for j in range(kg):
                            wt = wp.tile([128, 2, W], F16, tag="w",
                                         name=f"w{k0 + j}")
                            if dmasplit == 1:
                                nc.sync.dma_start(wt, w2h[k0 + j])
                            else:
                                for c in range(2):
                                    nc.sync.dma_start(wt[:, c, :],
                                                      w2h[k0 + j][:, c * W:
                                                                  (c + 1) * W])
                            wts.append(wt)
                    if "ybuild" in ablate:
                        y8 = yconst
                    else:
                        hbc = yb.tile([128, kg, BH], F16, tag="hbc")
                        for j in range(kg):
                            ksl = slice((k0 + j) * BH, (k0 + j + 1) * BH)
                            if j < npool:
                                nc.gpsimd.partition_broadcast(
                                    hbc[:, j, :], hTf[0:1, ksl])
                            else:
                                pt = mps.tile([128, BH], F32, tag="pt")
                                nc.tensor.matmul(pt, ones16, hTf[0:1, ksl],
                                                 start=True, stop=True)
                                nc.scalar.copy(hbc[:, j, :], pt)
                        y8 = yb.tile([128, kg, 2, BH], F16, tag="y")
                        hw = kg // 2
                        for g in range(2):
                            gs = slice(g * hw, (g + 1) * hw)
                            in0 = sxh[:, :, :].unsqueeze(1).broadcast_to(
                                (128, hw, 2, BH))
                            in1 = hbc[:, gs, :].unsqueeze(2).broadcast_to(
                                (128, hw, 2, BH))
                            nc.vector.tensor_tensor(y8[:, gs, :, :], in0, in1,
                                                    op=OP.mult)
                    if "mm" not in ablate:
                        mm = nc.tensor.matmul
                        for j in range(kg):
                            for c in range(2):
                                for bt in range(NBT):
                                    bs = slice(bt * 128, bt * 128 + 128)
                                    mm(psA[bt], y8[:, j, c, bs],
                                       wts[j][:, c, :], start=False,
                                       stop=False, skip_group_check=True)

              if repeat_main == 1:
                  _main_body()
              else:
                  with tc.For_i(0, repeat_main, 1):
                      _main_body()

              # close the accumulation groups
              for bt in range(NBT):
                  nc.tensor.matmul(psA[bt], ones16, zc, start=False,
                                   stop=True, skip_group_check=True)

            # ---------------- epilogue: sin/cos/relu, gate, store ----------
            def sin_reduced(v, outname):
                """sin(v) via range reduction robust to trunc- or
                round-to-nearest float->int conversion."""
                t1 = tp.tile([128, DP], F32, tag="t1")
                nc.vector.tensor_scalar_mul(t1, v, INV2PI)
                ti = tp.tile([128, DP], dt.int32, tag="ti")
                nc.vector.tensor_copy(ti, t1)
                tf = tp.tile([128, DP], F32, tag="tf")
                nc.vector.tensor_copy(tf, ti)
                r = tp.tile([128, DP], F32, tag="r")
                nc.vector.scalar_tensor_tensor(r, tf, N2PI, v,
                                               op0=OP.mult, op1=OP.add)
                m = tp.tile([128, DP], F32, tag="m")
                nc.vector.tensor_scalar(m, r, math.pi, None, op0=OP.is_gt)
                nc.vector.scalar_tensor_tensor(r, m, N2PI, r,
                                               op0=OP.mult, op1=OP.add)
                nc.vector.tensor_scalar(m, r, -math.pi, None, op0=OP.is_lt)
                nc.vector.scalar_tensor_tensor(r, m, -N2PI, r,
                                               op0=OP.mult, op1=OP.add)
                sv = tp.tile([128, DP], F32, tag=outname, name=outname)
                nc.scalar.activation(sv, r, AF.Sin)
                return sv

            for bt in range(NBT):
                for e in range(2):
                    th = psA[bt][:, e * WE:e * WE + DP]
                    g = gw_sb[bt][:, e:e + 1]

                    sv = sin_reduced(th, "sv")
                    u = tp.tile([128, DP], F32, tag="u")
                    nc.vector.tensor_scalar_add(u, th, math.pi / 2)
                    cv = sin_reduced(u, "cv")

                    nn = tp.tile([128, DN], F32, tag="nn")
                    nc.scalar.activation(
                        nn, psA[bt][:, e * WE + DP:(e + 1) * WE], AF.Relu)

                    if e == 0:
                        nc.vector.tensor_scalar_mul(outf[bt][:, 0:DP], cv, g)
                        nc.vector.tensor_scalar_mul(outf[bt][:, DP:2 * DP],
                                                    sv, g)
                        nc.vector.tensor_scalar_mul(outf[bt][:, 2 * DP:OUT],
                                                    nn, g)
                    else:
                        stt = nc.vector.scalar_tensor_tensor
                        stt(outf[bt][:, 0:DP], cv, g, outf[bt][:, 0:DP],
                            op0=OP.mult, op1=OP.add)
                        stt(outf[bt][:, DP:2 * DP], sv, g,
                            outf[bt][:, DP:2 * DP], op0=OP.mult, op1=OP.add)
                        stt(outf[bt][:, 2 * DP:OUT], nn, g,
                            outf[bt][:, 2 * DP:OUT], op0=OP.mult, op1=OP.add)
                nc.sync.dma_start(out[bt * 128:bt * 128 + 128, :], outf[bt])

    nc.finalize()
    return nc


def _host_prep(x, cond, base_wp, base_wn, base_bn, hW1, hb1, hW2, hb2,
               gW1, gb1, gW2, gb2, terms=1):
    """Build the 8 per-core input maps (layout prep + sharding only)."""
    f32 = np.float32
    f16 = np.float16
    W2r = np.asarray(hW2, f32).reshape(H, N, TPE)
    hb2r = np.asarray(hb2, f32).reshape(N, TPE)
    hwp = hb2r[:, :IN * DP].reshape(N, IN, DP)
    hwn = hb2r[:, IN * DP:IN * DP + IN * DN].reshape(N, IN, DN)
    hbn = hb2r[:, IN * DP + IN * DN:]                      # (N, DN)

    base_wp = np.asarray(base_wp, f32)
    base_wn = np.asarray(base_wn, f32)
    base_bn = np.asarray(base_bn, f32)
    x = np.asarray(x, f32)
    cond = np.asarray(cond, f32)
    gW2 = np.asarray(gW2, f32)
    gb2 = np.asarray(gb2, f32)

    common = dict(
        hW1=np.ascontiguousarray(np.asarray(hW1, f16)),
        hb1=np.asarray(hb1, f16).reshape(1, H).copy(),
        gW1=np.ascontiguousarray(np.asarray(gW1, f16)),
        gb1=np.asarray(gb1, f32).reshape(3 * N, 1).copy(),
    )

    halves = []
    for hb in range(2):
        bs = slice(hb * BH, (hb + 1) * BH)
        xT = np.ascontiguousarray(x[bs].T).reshape(2, 128, BH)
        halves.append(dict(
            xT32=np.ascontiguousarray(xT),
            xh16=np.ascontiguousarray(xT.astype(f16)),
            condT=np.ascontiguousarray(cond[bs].T.astype(f16)),
        ))

    pairs = []
    for p in range(4):
        e0, e1 = 2 * p, 2 * p + 1
        blocks = []
        for c in range(2):
            cs = slice(c * 128, (c + 1) * 128)
            blk = np.concatenate([
                W2r[:, e0, :IN * DP].reshape(H, IN, DP)[:, cs],
                W2r[:, e0, IN * DP:IN * DP + IN * DN].reshape(
                    H, IN, DN)[:, cs],
                W2r[:, e1, :IN * DP].reshape(H, IN, DP)[:, cs],
                W2r[:, e1, IN * DP:IN * DP + IN * DN].reshape(
                    H, IN, DN)[:, cs],
            ], axis=-1)                                     # (H, 128, 384)
            blocks.append(blk)
        w2h = np.concatenate(blocks, axis=-1).astype(f16)   # (H, 128, 768)
        w2b = np.stack([
            np.concatenate([W2r[:, e, IN * DP + IN * DN:],
                            (base_bn[e] + hbn[e])[None, :]], axis=0)
            for e in (e0, e1)], axis=1).astype(f16)         # (65, 2, 128)
        wb = np.concatenate(
            [base_wp[e0] + hwp[e0], base_wn[e0] + hwn[e0],
             base_wp[e1] + hwp[e1], base_wn[e1] + hwn[e1]],
            axis=-1)                                        # (IN, 384)
        perm = [e0, e1] + [j for j in range(N) if j not in (e0, e1)]
        pairs.append(dict(
            w2h=np.ascontiguousarray(w2h),
            w2b=np.ascontiguousarray(w2b),
            wbase=np.ascontiguousarray(wb.reshape(2, 128, W)),
            gW2=np.ascontiguousarray(gW2[:, perm].astype(f16)),
            gb2=np.ascontiguousarray(gb2[perm].reshape(1, N).astype(f16)),
        ))

    in_maps = []
    for c in range(8):
        p, hb = c // 2, c % 2
        m = dict(common)
        m.update(halves[hb])
        m.update(pairs[p])
        in_maps.append(m)
    return in_maps


def _make_runner(nc, n_cores=8):
    """Compile once; reusable executor for per-core input maps."""
    import jax
    from jax.sharding import Mesh, PartitionSpec
    from jax.experimental.shard_map import shard_map
    from concourse.bass2jax import (_bass_exec_p, install_neuronx_cc_hook,
                                    partition_id_tensor)

    install_neuronx_cc_hook()
    pname = nc.partition_id_tensor.name if nc.partition_id_tensor else None
    in_names, out_names, out_avals, zero_outs = [], [], [], []
    for alloc in nc.m.functions[0].allocations:
        if not isinstance(alloc, mybir.MemoryLocationSet):
            continue
        name = alloc.memorylocations[0].name
        if alloc.kind == "ExternalInput":
            if name != pname:
                in_names.append(name)
        elif alloc.kind == "ExternalOutput":
            out_names.append(name)
            shape = tuple(alloc.tensor_shape)
            dtype = mybir.dt.np(alloc.dtype)
            out_avals.append(jax.core.ShapedArray(shape, dtype))
            zero_outs.append(np.zeros(shape, dtype))
    n_params = len(in_names)
    n_outs = len(out_avals)
    all_names = in_names + out_names + ([pname] if pname else [])

    def _body(*args):
        operands = list(args)
        if pname is not None:
            operands.append(partition_id_tensor())
        outs = _bass_exec_p.bind(
            *operands, out_avals=tuple(out_avals), in_names=tuple(all_names),
            out_names=tuple(out_names), lowering_input_output_aliases=(),
            sim_require_finite=True, sim_require_nnan=True, nc=nc)
        return tuple(outs)

    devices = jax.devices()[:n_cores]
    mesh = Mesh(np.asarray(devices), ("core",))
    in_specs = (PartitionSpec("core"),) * (n_params + n_outs)
    out_specs = (PartitionSpec("core"),) * n_outs
    donate = tuple(range(n_params, n_params + n_outs))
    sharded = jax.jit(
        shard_map(_body, mesh=mesh, in_specs=in_specs, out_specs=out_specs,
                  check_rep=False),
        donate_argnums=donate, keep_unused=True)

    staged = {}

    def _concat(in_maps):
        return [
            np.concatenate([np.asarray(in_maps[c][in_names[i]])
                            for c in range(n_cores)], axis=0)
            for i in range(n_params)
        ]

    def run(in_maps):
        if in_maps is None:
            concat_in = staged["dev"]
        else:
            concat_in = _concat(in_maps)
        zeros = [np.zeros((n_cores * z.shape[0], *z.shape[1:]), z.dtype)
                 for z in zero_outs]
        outs = sharded(*concat_in, *zeros)
        arr = np.asarray(outs[0]).reshape(n_cores, *out_avals[0].shape)
        return [{out_names[0]: arr[c]} for c in range(n_cores)]

    def preload(in_maps):
        import jax
        staged["dev"] = [jax.device_put(a) for a in _concat(in_maps)]
        for a in staged["dev"]:
            a.block_until_ready()

    run.preload = preload
    return run


def kernel(**inputs):
    terms = _cache.setdefault("terms", 1)
    if "run" not in _cache:
        nc = _build(terms)
        _cache["nc"] = nc
        _cache["run"] = _make_runner(nc)
    in_maps = _host_prep(**inputs, terms=terms)
    results = _cache["run"](in_maps)
    out = np.zeros((B, OUT), np.float32)
    for c in range(8):
        hb = c % 2
        out[hb * BH:(hb + 1) * BH] += results[c]["out"]
    return out


# revision 15
# speedup vs baseline: 1.0040x; 1.0040x over previous
"""FANMoE HyperNet layer on 8 TRN2 NeuronCores — pair-sharded, y-stationary.

Sharding: 4 expert-pairs x 2 batch-halves. Each core owns 2 experts
(W = 2*(DP+DN) = 384 output cols) and 256 samples, streaming its fp16
hW2 hi-slice (12.6MB) from HBM once per call.

Key structure: the per-k combine out += h[:,k] * (x @ W_k) is folded into
the matmul by scaling the stationary operand: y_k[i,b] = x_hi[i,b]*h[k,b]
(fp16, built on DVE with broadcast APs from a partition-broadcast of h).
PSUM then accumulates all 64 k plus the base/bias terms in one
accumulation group per batch tile — no per-k elementwise combine exists
(measured: any such combine costs >=51us on the fastest engine).

Measured-engine budget per core (main loop): PE 256 mms x 384 rows x
165ns = 42us; DMA weights 12.6MB ~ 49us (the wall); DVE grouped y-build
~19us; h-broadcast split Pool (757ns/k) / PE+Act ones-matmul path.

Accuracy: 1-term fp16 (x_hi, h_16, W_hi, y rounded) gives ~2^-11-relative
matmul error => phase error ~4e-3 rms on per-values, ~5e-3 relative
output error against the 2e-2 gate.
"""
import math

import numpy as np

import concourse.bass as bass
import concourse.tile as tile
from concourse import mybir, bacc
from concourse.masks import make_identity

B, IN, OUT, COND, N, H = 512, 256, 256, 128, 8, 64
DP = 64
DN = 128
TPE = IN * DP + IN * DN + DN
BH = B // 2          # samples per core (batch half)
NBT = BH // 128      # 2 tiles of 128 rows
WE = DP + DN         # per-expert width 192
W = 2 * WE           # per-core output width 384
dt = mybir.dt
F32 = dt.float32
F16 = dt.float16
AF = mybir.ActivationFunctionType
OP = mybir.AluOpType
INV2PI = 1.0 / (2.0 * math.pi)
N2PI = -2.0 * math.pi

_cache = {}

KG = 8        # k-group size (DMA/y-build granularity)
NPOOL = 8     # of each KG: first NPOOL h-broadcasts on Pool, rest PE+Act
DMASPLIT = 1  # weight-tile DMAs issued per k (1 or 2 halves)


def _build(terms=1, repeat_main=1, ablate=(), kg=None, npool=None,
           dmasplit=None):
    kg = KG if kg is None else kg
    npool = NPOOL if npool is None else npool
    dmasplit = DMASPLIT if dmasplit is None else dmasplit
    nc = bacc.Bacc("TRN2", target_bir_lowering=False, debug=False)

    def din(name, shape, dty=F32):
        return nc.dram_tensor(name, shape, dty, kind="ExternalInput").ap()

    xT32 = din("xT32", (2, 128, BH))
    xh16 = din("xh16", (2, 128, BH), F16)
    w2h = din("w2h", (H, 128, 2 * W), F16)
    w2b = din("w2b", (H + 1, 2, DN), F16)
    wbase = din("wbase", (2, 128, W))
    condT = din("condT", (COND, BH), F16)
    hW1 = din("hW1", (COND, H), F16)
    hb1 = din("hb1", (1, H), F16)
    gW1 = din("gW1", (COND, 3 * N), F16)
    gb1 = din("gb1", (3 * N, 1))
    gW2 = din("gW2", (3 * N, N), F16)
    gb2 = din("gb2", (1, N), F16)
    out = nc.dram_tensor("out", (BH, OUT), F32, kind="ExternalOutput").ap()

    with tile.TileContext(nc) as tc:
        with tc.tile_pool(name="const", bufs=1) as cp, \
             tc.tile_pool(name="tmp", bufs=4) as tp, \
             tc.tile_pool(name="acc", bufs=1, space="PSUM") as ap_:
            ident = cp.tile([128, 128], F16)
            make_identity(nc, ident)
            ones16 = cp.tile([1, 128], F16)
            nc.vector.memset(ones16, 1.0)
            zc = cp.tile([1, W], F16)
            nc.vector.memset(zc, 0.0)

            psA = [ap_.tile([128, W], F32, tag=f"psA{t}", name=f"psA{t}",
                            bufs=1) for t in range(NBT)]

            sxT = cp.tile([128, 2, BH], F32)
            sxh = cp.tile([128, 2, BH], F16)
            for c in range(2):
                nc.sync.dma_start(sxT[:, c, :], xT32[c])
                nc.sync.dma_start(sxh[:, c, :], xh16[c])
            scT = cp.tile([COND, BH], F16)
            nc.sync.dma_start(scT, condT)
            swb = cp.tile([128, 2, W], F32)
            for c in range(2):
                nc.sync.dma_start(swb[:, c, :], wbase[c])
            sw2b = cp.tile([H + 1, 2, DN], F16)
            for e in range(2):
                nc.sync.dma_start(sw2b[:, e, :], w2b[:, e])
            shW1 = cp.tile([COND, H], F16)
            nc.sync.dma_start(shW1, hW1)
            shb1 = cp.tile([1, H], F16)
            nc.sync.dma_start(shb1, hb1)
            sgW1 = cp.tile([COND, 3 * N], F16)
            nc.sync.dma_start(sgW1, gW1)
            sgb1 = cp.tile([3 * N, 1], F32)
            nc.sync.dma_start(sgb1, gb1)
            sgW2 = cp.tile([3 * N, N], F16)
            nc.sync.dma_start(sgW2, gW2)
            sgb2 = cp.tile([1, N], F16)
            nc.sync.dma_start(sgb2, gb2)

            hTa = cp.tile([H + 1, BH], F16)
            nc.vector.memset(hTa[H:H + 1, :], 1.0)
            hTf = cp.tile([1, H * BH], F16)
            h16 = [cp.tile([128, H], F16, name=f"h{t}") for t in range(NBT)]
            outf = [cp.tile([128, OUT], F32, name=f"of{t}") for t in range(NBT)]
            gw_sb = [cp.tile([128, 2], F32, name=f"gw{t}") for t in range(NBT)]

            # ---------------- prologue: gating, hypernet h, base ----------
            with tc.tile_pool(name="pps", bufs=2, space="PSUM") as pps:
                g1 = pps.tile([3 * N, BH], F32, tag="g1", bufs=1)
                nc.tensor.matmul(g1, sgW1, scT, start=True, stop=True)
                g1s = cp.tile([3 * N, BH], F16)
                nc.scalar.activation(g1s, g1, AF.Relu, bias=sgb1)

                for bt in range(NBT):
                    bs = slice(bt * 128, bt * 128 + 128)
                    hp = pps.tile([128, H], F32, tag="hp", bufs=1)
                    nc.tensor.matmul(hp, scT[:, bs], shW1,
                                     start=True, stop=False)
                    nc.tensor.matmul(hp, ones16, shb1, start=False, stop=True)
                    nc.scalar.activation(h16[bt], hp, AF.Relu)
                    ht = pps.tile([H, 128], F16, tag="ht", bufs=1)
                    nc.tensor.transpose(ht, h16[bt], ident)
                    nc.scalar.copy(hTa[0:H, bs], ht)

                    lg = pps.tile([128, N], F32, tag="lg", bufs=1)
                    nc.tensor.matmul(lg, g1s[:, bs], sgW2,
                                     start=True, stop=False)
                    nc.tensor.matmul(lg, ones16, sgb2, start=False, stop=True)
                    nmx = tp.tile([128, 1], F32, tag="nmx")
                    nc.vector.tensor_reduce(nmx, lg, axis=mybir.AxisListType.X,
                                            op=OP.max, negate=True)
                    ex = tp.tile([128, N], F32, tag="ex")
                    nc.scalar.activation(ex, lg, AF.Exp, bias=nmx)
                    sm = tp.tile([128, 1], F32, tag="sm")
                    nc.vector.tensor_reduce(sm, ex, axis=mybir.AxisListType.X,
                                            op=OP.add)
                    rv = tp.tile([128, 1], F32, tag="rv")
                    nc.vector.reciprocal(rv, sm)
                    nc.vector.tensor_scalar_mul(gw_sb[bt], ex[:, 0:2], rv)

                    # open the psA accumulation group: base + biases
                    nc.tensor.matmul(psA[bt], sxT[:, 0, bs], swb[:, 0, :],
                                     start=True, stop=False,
                                     skip_group_check=True)
                    nc.tensor.matmul(psA[bt], sxT[:, 1, bs], swb[:, 1, :],
                                     start=False, stop=False,
                                     skip_group_check=True)
                    for e in range(2):
                        cs = slice(e * WE + DP, (e + 1) * WE)
                        nc.tensor.matmul(psA[bt][:, cs], hTa[:, bs],
                                         sw2b[:, e, :], start=False,
                                         stop=False, skip_group_check=True)

                # h rows -> one partition-0 row for partition_broadcast
                for k in range(H):
                    nc.sync.dma_start(hTf[0:1, k * BH:(k + 1) * BH],
                                      hTa[k:k + 1, :])

            # ---------------- main loop over hypernet units k --------------
            with tc.tile_pool(name="wp", bufs=2 * kg) as wp, \
                 tc.tile_pool(name="yb", bufs=2) as yb, \
                 tc.tile_pool(name="mps", bufs=2, space="PSUM") as mps:

              wconst = None
              if "dma" in ablate:
                  wconst = [cp.tile([128, 2, W], F16, name=f"wc{i}")
                            for i in range(kg)]
                  for i in range(kg):
                      nc.sync.dma_start(wconst[i], w2h[i])
              yconst = hconst = None
              if "ybuild" in ablate:
                  yconst = cp.tile([128, kg, 2, 128 * NBT], F16, name="yc")
                  nc.vector.memset(yconst, 0.25)

              def _main_body():
                for k0 in range(0, H, kg):
                    if "dma" in ablate:
                        wts = wconst
                    else:
                        wts = []
                        for j in range(kg):
                            wt = wp.tile([128, 2, W], F16, tag="w",
                                         name=f"w{k0 + j}")
                            if dmasplit == 1:
                                nc.sync.dma_start(wt, w2h[k0 + j])
                            else:
                                for c in range(2):
                                    nc.sync.dma_start(wt[:, c, :],
                                                      w2h[k0 + j][:, c * W:
                                                                  (c + 1) * W])
                            wts.append(wt)
                    if "ybuild" in ablate:
                        y8 = yconst
                    else:
                        hbc = yb.tile([128, kg, BH], F16, tag="hbc")
                        for j in range(kg):
                            ksl = slice((k0 + j) * BH, (k0 + j + 1) * BH)
                            if j < npool:
                                nc.gpsimd.partition_broadcast(
                                    hbc[:, j, :], hTf[0:1, ksl])
                            else:
                                pt = mps.tile([128, BH], F32, tag="pt")
                                nc.tensor.matmul(pt, ones16, hTf[0:1, ksl],
                                                 start=True, stop=True)
                                nc.scalar.copy(hbc[:, j, :], pt)
                        y8 = yb.tile([128, kg, 2, BH], F16, tag="y")
                        hw = kg // 2
                        for g in range(2):
                            gs = slice(g * hw, (g + 1) * hw)
                            in0 = sxh[:, :, :].unsqueeze(1).broadcast_to(
                                (128, hw, 2, BH))
                            in1 = hbc[:, gs, :].unsqueeze(2).broadcast_to(
                                (128, hw, 2, BH))
                            nc.vector.tensor_tensor(y8[:, gs, :, :], in0, in1,
                                                    op=OP.mult)
                    if "mm" not in ablate:
                        mm = nc.tensor.matmul
                        for j in range(kg):
                            for c in range(2):
                                for bt in range(NBT):
                                    bs = slice(bt * 128, bt * 128 + 128)
                                    mm(psA[bt], y8[:, j, c, bs],
                                       wts[j][:, c, :], start=False,
                                       stop=False, skip_group_check=True)

              if repeat_main == 1:
                  _main_body()
              else:
                  with tc.For_i(0, repeat_main, 1):
                      _main_body()

              # close the accumulation groups
              for bt in range(NBT):
                  nc.tensor.matmul(psA[bt], ones16, zc, start=False,
                                   stop=True, skip_group_check=True)

            # ---------------- epilogue: sin/cos/relu, gate, store ----------
            def sin_reduced(v, outname):
                """sin(v) via range reduction robust to trunc- or
                round-to-nearest float->int conversion."""
                t1 = tp.tile([128, DP], F32, tag="t1")
                nc.vector.tensor_scalar_mul(t1, v, INV2PI)
                ti = tp.tile([128, DP], dt.int32, tag="ti")
                nc.vector.tensor_copy(ti, t1)
                tf = tp.tile([128, DP], F32, tag="tf")
                nc.vector.tensor_copy(tf, ti)
                r = tp.tile([128, DP], F32, tag="r")
                nc.vector.scalar_tensor_tensor(r, tf, N2PI, v,
                                               op0=OP.mult, op1=OP.add)
                m = tp.tile([128, DP], F32, tag="m")
                nc.vector.tensor_scalar(m, r, math.pi, None, op0=OP.is_gt)
                nc.vector.scalar_tensor_tensor(r, m, N2PI, r,
                                               op0=OP.mult, op1=OP.add)
                nc.vector.tensor_scalar(m, r, -math.pi, None, op0=OP.is_lt)
                nc.vector.scalar_tensor_tensor(r, m, -N2PI, r,
                                               op0=OP.mult, op1=OP.add)
                sv = tp.tile([128, DP], F32, tag=outname, name=outname)
                nc.scalar.activation(sv, r, AF.Sin)
                return sv

            for bt in range(NBT):
                for e in range(2):
                    th = psA[bt][:, e * WE:e * WE + DP]
                    g = gw_sb[bt][:, e:e + 1]

                    sv = sin_reduced(th, "sv")
                    u = tp.tile([128, DP], F32, tag="u")
                    nc.vector.tensor_scalar_add(u, th, math.pi / 2)
                    cv = sin_reduced(u, "cv")

                    nn = tp.tile([128, DN], F32, tag="nn")
                    nc.scalar.activation(
                        nn, psA[bt][:, e * WE + DP:(e + 1) * WE], AF.Relu)

                    if e == 0:
                        nc.vector.tensor_scalar_mul(outf[bt][:, 0:DP], cv, g)
                        nc.vector.tensor_scalar_mul(outf[bt][:, DP:2 * DP],
                                                    sv, g)
                        nc.vector.tensor_scalar_mul(outf[bt][:, 2 * DP:OUT],
                                                    nn, g)
                    else:
                        stt = nc.vector.scalar_tensor_tensor
                        stt(outf[bt][:, 0:DP], cv, g, outf[bt][:, 0:DP],
                            op0=OP.mult, op1=OP.add)
                        stt(outf[bt][:, DP:2 * DP], sv, g,
                            outf[bt][:, DP:2 * DP], op0=OP.mult, op1=OP.add)
                        stt(outf[bt][:, 2 * DP:OUT], nn, g,
                            outf[bt][:, 2 * DP:OUT], op0=OP.mult, op1=OP.add)
                nc.sync.dma_start(out[bt * 128:bt * 128 + 128, :], outf[bt])

    nc.finalize()
    return nc


def _host_prep(x, cond, base_wp, base_wn, base_bn, hW1, hb1, hW2, hb2,
               gW1, gb1, gW2, gb2, terms=1):
    """Build the 8 per-core input maps (layout prep + sharding only)."""
    f32 = np.float32
    f16 = np.float16
    W2r = np.asarray(hW2, f32).reshape(H, N, TPE)
    hb2r = np.asarray(hb2, f32).reshape(N, TPE)
    hwp = hb2r[:, :IN * DP].reshape(N, IN, DP)
    hwn = hb2r[:, IN * DP:IN * DP + IN * DN].reshape(N, IN, DN)
    hbn = hb2r[:, IN * DP + IN * DN:]                      # (N, DN)

    base_wp = np.asarray(base_wp, f32)
    base_wn = np.asarray(base_wn, f32)
    base_bn = np.asarray(base_bn, f32)
    x = np.asarray(x, f32)
    cond = np.asarray(cond, f32)
    gW2 = np.asarray(gW2, f32)
    gb2 = np.asarray(gb2, f32)

    common = dict(
        hW1=np.ascontiguousarray(np.asarray(hW1, f16)),
        hb1=np.asarray(hb1, f16).reshape(1, H).copy(),
        gW1=np.ascontiguousarray(np.asarray(gW1, f16)),
        gb1=np.asarray(gb1, f32).reshape(3 * N, 1).copy(),
    )

    halves = []
    for hb in range(2):
        bs = slice(hb * BH, (hb + 1) * BH)
        xT = np.ascontiguousarray(x[bs].T).reshape(2, 128, BH)
        halves.append(dict(
            xT32=np.ascontiguousarray(xT),
            xh16=np.ascontiguousarray(xT.astype(f16)),
            condT=np.ascontiguousarray(cond[bs].T.astype(f16)),
        ))

    pairs = []
    for p in range(4):
        e0, e1 = 2 * p, 2 * p + 1
        blocks = []
        for c in range(2):
            cs = slice(c * 128, (c + 1) * 128)
            blk = np.concatenate([
                W2r[:, e0, :IN * DP].reshape(H, IN, DP)[:, cs],
                W2r[:, e0, IN * DP:IN * DP + IN * DN].reshape(
                    H, IN, DN)[:, cs],
                W2r[:, e1, :IN * DP].reshape(H, IN, DP)[:, cs],
                W2r[:, e1, IN * DP:IN * DP + IN * DN].reshape(
                    H, IN, DN)[:, cs],
            ], axis=-1)                                     # (H, 128, 384)
            blocks.append(blk)
        w2h = np.concatenate(blocks, axis=-1).astype(f16)   # (H, 128, 768)
        w2b = np.stack([
            np.concatenate([W2r[:, e, IN * DP + IN * DN:],
                            (base_bn[e] + hbn[e])[None, :]], axis=0)
            for e in (e0, e1)], axis=1).astype(f16)         # (65, 2, 128)
        wb = np.concatenate(
            [base_wp[e0] + hwp[e0], base_wn[e0] + hwn[e0],
             base_wp[e1] + hwp[e1], base_wn[e1] + hwn[e1]],
            axis=-1)                                        # (IN, 384)
        perm = [e0, e1] + [j for j in range(N) if j not in (e0, e1)]
        pairs.append(dict(
            w2h=np.ascontiguousarray(w2h),
            w2b=np.ascontiguousarray(w2b),
            wbase=np.ascontiguousarray(wb.reshape(2, 128, W)),
            gW2=np.ascontiguousarray(gW2[:, perm].astype(f16)),
            gb2=np.ascontiguousarray(gb2[perm].reshape(1, N).astype(f16)),
        ))

    in_maps = []
    for c in range(8):
        p, hb = c // 2, c % 2
        m = dict(common)
        m.update(halves[hb])
        m.update(pairs[p])
        in_maps.append(m)
    return in_maps


def _make_runner(nc, n_cores=8):
    """Compile once; reusable executor for per-core input maps."""
    import jax
    from jax.sharding import Mesh, PartitionSpec
    from jax.experimental.shard_map import shard_map
    from concourse.bass2jax import (_bass_exec_p, install_neuronx_cc_hook,
                                    partition_id_tensor)

    install_neuronx_cc_hook()
    pname = nc.partition_id_tensor.name if nc.partition_id_tensor else None
    in_names, out_names, out_avals, zero_outs = [], [], [], []
    for alloc in nc.m.functions[0].allocations:
        if not isinstance(alloc, mybir.MemoryLocationSet):
            continue
        name = alloc.memorylocations[0].name
        if alloc.kind == "ExternalInput":
            if name != pname:
                in_names.append(name)
        elif alloc.kind == "ExternalOutput":
            out_names.append(name)
            shape = tuple(alloc.tensor_shape)
            dtype = mybir.dt.np(alloc.dtype)
            out_avals.append(jax.core.ShapedArray(shape, dtype))
            zero_outs.append(np.zeros(shape, dtype))
    n_params = len(in_names)
    n_outs = len(out_avals)
    all_names = in_names + out_names + ([pname] if pname else [])

    def _body(*args):
        operands = list(args)
        if pname is not None:
            operands.append(partition_id_tensor())
        outs = _bass_exec_p.bind(
            *operands, out_avals=tuple(out_avals), in_names=tuple(all_names),
            out_names=tuple(out_names), lowering_input_output_aliases=(),
            sim_require_finite=True, sim_require_nnan=True, nc=nc)
        return tuple(outs)

    devices = jax.devices()[:n_cores]
    mesh = Mesh(np.asarray(devices), ("core",))
    in_specs = (PartitionSpec("core"),) * (n_params + n_outs)
    out_specs = (PartitionSpec("core"),) * n_outs
    donate = tuple(range(n_params, n_params + n_outs))
    sharded = jax.jit(
        shard_map(_body, mesh=mesh, in_specs=in_specs, out_specs=out_specs,
                  check_rep=False),
        donate_argnums=donate, keep_unused=True)

    staged = {}

    def _concat(in_maps):
        return [
            np.concatenate([np.asarray(in_maps[c][in_names[i]])
                            for c in range(n_cores)], axis=0)
            for i in range(n_params)
        ]

    def run(in_maps):
        if in_maps is None:
            concat_in = staged["dev"]
        else:
            concat_in = _concat(in_maps)
        zeros = [np.zeros((n_cores * z.shape[0], *z.shape[1:]), z.dtype)
                 for z in zero_outs]
        outs = sharded(*concat_in, *zeros)
        arr = np.asarray(outs[0]).reshape(n_cores, *out_avals[0].shape)
        return [{out_names[0]: arr[c]} for c in range(n_cores)]

    def preload(in_maps):
        import jax
        staged["dev"] = [jax.device_put(a) for a in _concat(in_maps)]
        for a in staged["dev"]:
            a.block_until_ready()

    run.preload = preload
    return run


def kernel(**inputs):
    terms = _cache.setdefault("terms", 1)
    if "run" not in _cache:
        nc = _build(terms)
        _cache["nc"] = nc
        _cache["run"] = _make_runner(nc)
    in_maps = _host_prep(**inputs, terms=terms)
    results = _cache["run"](in_maps)
    out = np.zeros((B, OUT), np.float32)
    for c in range(8):
        hb = c % 2
        out[hb * BH:(hb + 1) * BH] += results[c]["out"]
    return out
